# revision 1
# baseline (speedup 1.0000x reference)
"""Causal self-attention on 8 Trainium2 NeuronCores.

Reference (fp32):
    qkv = x @ W_qkv + b_qkv ; split q,k,v ; heads H=16, Dh=64
    scores = q @ k^T / sqrt(Dh), causal mask, softmax
    out = (attn @ v) re-merged ; y = out @ W_proj + b_proj

Sharding: tensor-parallel over heads x data-parallel over batch.
Core c (0..7) owns batch b = c//4 and head group g = c%4 (heads 4g..4g+3).
Each core computes q^T,k^T,v for its 4 heads from x[b]^T, runs causal
attention (scores transposed layout, exp without max-subtraction -- scores
are O(5) so fp32 exp is safe, denominator via an appended ones-column in
the V matmul), then its partial y^T = O^T @ W_proj[rows]. The 4 cores of a
batch ReduceScatter(add) the [1024, 2048] partial y^T in 4 row chunks
overlapped with the projection; each core adds its b_proj slice and
returns 4 x [64, 2048] row-slices of y^T. Host reassembles.

Matmuls run as float32r (reduced-precision fp32, 4x faster than fp32 on
the PE); end-to-end error vs the fp32 reference is ~3e-4 of max|y|.
The two heads of a pair occupy PE rows 0:64 / 64:128 so their score
matmuls execute concurrently in disjoint row groups.
"""

import numpy as np

import concourse.bacc as bacc
import concourse.mybir as mybir
import concourse.tile as tile
from concourse.bass_utils import run_bass_kernel_spmd

B = 2
T = 2048
C = 1024
H = 16
DH = 64
G = 4  # heads per core
N_CORES = 8
TQ = 512  # q-chunk width
NKT = T // 128  # k tiles per head
NJQ = T // TQ  # q chunks
NCK = C // 128  # contraction tiles over model dim
SCALE = 1.0 / np.sqrt(DH)
GROUPS = [[0, 1, 2, 3], [4, 5, 6, 7]]

F32 = mybir.dt.float32
F32R = mybir.dt.float32r
BF16 = mybir.dt.bfloat16
FP16 = mybir.dt.float16
# operand dtype for all the big matmuls (fp16: full speed + fast weight
# load like bf16, but 8x finer mantissa; all values here fit fp16 range)
ATT_DT = FP16
MM_DT = FP16

_PROG = None


def _build_program():
    nc = bacc.Bacc(
        "TRN2", target_bir_lowering=False, debug=False, num_devices=N_CORES
    )
    xt_d = nc.dram_tensor("xt", [C, T], MM_DT, kind="ExternalInput").ap()
    wq_d = nc.dram_tensor("wq", [C, G * DH], MM_DT, kind="ExternalInput").ap()
    wk_d = nc.dram_tensor("wk", [C, G * DH], MM_DT, kind="ExternalInput").ap()
    wv_d = nc.dram_tensor("wv", [C, G * DH], MM_DT, kind="ExternalInput").ap()
    wp_d = nc.dram_tensor("wp", [G * DH, C], MM_DT, kind="ExternalInput").ap()
    bq_d = nc.dram_tensor("bq", [G * DH, 1], F32, kind="ExternalInput").ap()
    bk_d = nc.dram_tensor("bk", [G * DH, 1], F32, kind="ExternalInput").ap()
    bv_d = nc.dram_tensor("bv", [1, G * DH], F32, kind="ExternalInput").ap()
    bp_d = nc.dram_tensor("bp", [C // 4, 1], F32, kind="ExternalInput").ap()
    mask_d = nc.dram_tensor("mask", [128, 896], ATT_DT, kind="ExternalInput").ap()
    ones_d = nc.dram_tensor("ones", [128, 64], F32R, kind="ExternalInput").ap()
    onesb_d = nc.dram_tensor("onesb", [128, 64], ATT_DT, kind="ExternalInput").ap()
    bc2_d = nc.dram_tensor("bc2", [2, 128], F32R, kind="ExternalInput").ap()
    rs_in = [nc.dram_tensor(f"rs_in{j}", [C, TQ], F32).ap() for j in range(NJQ)]
    rs_out = [nc.dram_tensor(f"rs_out{j}", [C // 4, TQ], F32).ap() for j in range(NJQ)]
    y_d = nc.dram_tensor("y", [C // 4, T], F32, kind="ExternalOutput").ap()

    with tile.TileContext(nc) as tc:
        with (
            nc.allow_low_precision(reason="float32r matmul pipeline by design"),
            tc.tile_pool(name="ll", bufs=1) as ll,
        ):
            # ---- long-lived tiles -------------------------------------
            qT = [ll.tile([128, T], ATT_DT, tag=f"qT{p}", name=f"qT{p}") for p in range(2)]
            kT = [ll.tile([128, T], ATT_DT, tag=f"kT{p}", name=f"kT{p}") for p in range(2)]
            oT = [ll.tile([128, T], ATT_DT, tag=f"oT{p}", name=f"oT{p}") for p in range(2)]
            vaug = [ll.tile([128, G * 65], ATT_DT, tag=f"va{t}", name=f"va{t}") for t in range(NKT)]


            mask = ll.tile([128, 896], ATT_DT, tag="mask")
            nc.sync.dma_start(out=mask[:], in_=mask_d[:])
            bc2_sb = ll.tile([2, 128], F32R, tag="bc2")
            nc.sync.dma_start(out=bc2_sb[:], in_=bc2_d[:])
            ones_sb = ll.tile([128, 64], F32R, tag="ones")
            nc.sync.dma_start(out=ones_sb[:], in_=ones_d[:])
            wp_sb = [ll.tile([128, C], MM_DT, tag=f"wp{p}", name=f"wp{p}") for p in range(2)]
            for p in range(2):
                nc.sync.dma_start(
                    out=wp_sb[p][:], in_=wp_d[p * 128 : (p + 1) * 128, :]
                )
            bq_sb = [ll.tile([128, 1], F32, tag=f"bq{p}", name=f"bq{p}") for p in range(2)]
            bk_sb = [ll.tile([128, 1], F32, tag=f"bk{p}", name=f"bk{p}") for p in range(2)]
            for p in range(2):
                nc.sync.dma_start(
                    out=bq_sb[p][:], in_=bq_d[p * 128 : (p + 1) * 128, :]
                )
                nc.sync.dma_start(
                    out=bk_sb[p][:], in_=bk_d[p * 128 : (p + 1) * 128, :]
                )
            bv_sb = ll.tile([1, G * DH], F32, tag="bv")
            nc.sync.dma_start(out=bv_sb[:], in_=bv_d[:])
            bp_sb = [ll.tile([128, 1], F32, tag=f"bp{i}", name=f"bp{i}") for i in range(2)]
            for i in range(2):
                nc.sync.dma_start(
                    out=bp_sb[i][:], in_=bp_d[i * 128 : (i + 1) * 128, :]
                )

            # ---- phase A: qkv projections -----------------------------
            with (
                tc.tile_pool(name="pa", bufs=1) as pa,
                tc.tile_pool(name="pamm", bufs=3, space="PSUM") as pamm,
            ):
                # bv broadcast across partitions (via ones-row matmul)
                ones_row = ll.tile([1, 128], F32R, tag="ones_row")
                nc.sync.dma_start(out=ones_row[:, 0:64], in_=ones_d[0:1, :])
                nc.sync.dma_start(out=ones_row[:, 64:128], in_=ones_d[0:1, :])
                bv_r = ll.tile([1, G * DH], F32R, tag="bvr")
                nc.vector.tensor_copy(out=bv_r[:], in_=bv_sb[:])
                bvb_ps = pamm.tile([128, G * DH], F32, tag="mm")
                bvb_sb = ll.tile([128, G * DH], F32, tag="bvb")
                nc.tensor.matmul(
                    bvb_ps[:], lhsT=ones_row[:], rhs=bv_r[:], start=True, stop=True
                )
                nc.vector.tensor_copy(out=bvb_sb[:], in_=bvb_ps[:])

                wq_sb, wk_sb, wv_sb = [], [], []
                xt_sb = [
                    pa.tile([128, T], MM_DT, tag=f"xt{k}", name=f"xt{k}")
                    for k in range(NCK)
                ]
                # j=0 column of xt + the weights, interleaved k-major on two
                # queues, so the first qkv chains unblock as early as possible
                for k in range(NCK):
                    nc.sync.dma_start(
                        out=xt_sb[k][:, 0:TQ],
                        in_=xt_d[k * 128 : (k + 1) * 128, 0:TQ],
                    )
                    for name, dst, src in (
                        ("q", wq_sb, wq_d),
                        ("k", wk_sb, wk_d),
                        ("v", wv_sb, wv_d),
                    ):
                        t = pa.tile([128, G * DH], MM_DT, tag=f"w{name}{k}", name=f"w{name}{k}")
                        nc.scalar.dma_start(
                            out=t[:], in_=src[k * 128 : (k + 1) * 128, :]
                        )
                        dst.append(t)
                for j in range(1, NJQ):
                    for k in range(NCK):
                        eng = (nc.sync, nc.scalar)[k % 2]
                        eng.dma_start(
                            out=xt_sb[k][:, j * TQ : (j + 1) * TQ],
                            in_=xt_d[k * 128 : (k + 1) * 128, j * TQ : (j + 1) * TQ],
                        )

                for j in range(NJQ):
                    # q^T / k^T chains for this column chunk
                    for wsb, bsb, dst in ((wq_sb, bq_sb, qT), (wk_sb, bk_sb, kT)):
                        for p in range(2):
                            ps = pamm.tile([128, TQ], F32, tag="mm")
                            for k in range(NCK):
                                nc.tensor.matmul(
                                    ps[:],
                                    lhsT=wsb[k][:, p * 128 : (p + 1) * 128],
                                    rhs=xt_sb[k][:, j * TQ : (j + 1) * TQ],
                                    start=(k == 0),
                                    stop=(k == NCK - 1),
                                )
                            nc.vector.tensor_scalar_add(
                                out=dst[p][:, j * TQ : (j + 1) * TQ],
                                in0=ps[:],
                                scalar1=bsb[p][:],
                            )
                    # v tiles covered by this column chunk
                    for t in range(4 * j, 4 * j + 4):
                        ps = pamm.tile([128, G * DH], F32, tag="mm")
                        for k in range(NCK):
                            nc.tensor.matmul(
                                ps[:],
                                lhsT=xt_sb[k][:, t * 128 : (t + 1) * 128],
                                rhs=wv_sb[k][:],
                                start=(k == 0),
                                stop=(k == NCK - 1),
                            )
                        va = vaug[t].rearrange("p (h x) -> p h x", x=65)
                        nc.vector.tensor_add(
                            out=va[:, :, 0:64],
                            in0=ps[:].rearrange("p (h x) -> p h x", x=64),
                            in1=bvb_sb[:].rearrange("p (h x) -> p h x", x=64),
                        )
                        nc.sync.dma_start(
                            out=va[:, :, 64:65],
                            in_=onesb_d[:, 0:G].rearrange("p (h x) -> p h x", x=1),
                        )

            # ---- phases B..D: attention, normalize, projection, RS ----
            # jq-outer so that normalize/proj/ReduceScatter pipeline per
            # 512-column block while later blocks still compute.
            with (
                tc.tile_pool(name="dt", bufs=4) as dtp,
                tc.tile_pool(name="rp", bufs=4) as rpp,
                tc.tile_pool(name="es", bufs=6) as esp,
                tc.tile_pool(name="oc", bufs=3) as ocp,
                tc.tile_pool(name="rsy", bufs=2) as rsyp,
                tc.tile_pool(name="ps0", bufs=2, space="PSUM") as sp0,
                tc.tile_pool(name="ov", bufs=4, space="PSUM") as ovp,
                tc.tile_pool(name="px", bufs=2, space="PSUM") as pxp,
            ):
                rp_map = {}

                def emit_attention(jq):
                    kmax = 4 * jq + 4
                    den4 = dtp.tile([4, TQ], F32, tag="den4", name="den4")
                    for p in range(2):
                        ov = [
                            ovp.tile([65, TQ], F32, tag="ov", name="ovA"),
                            ovp.tile([65, TQ], F32, tag="ov", name="ovB"),
                        ]
                        spool = (sp0, sp0)

                        def emit_v(kt, qlo, es_pair):
                            va = vaug[kt].rearrange("p (h x) -> p h x", x=65)
                            for half in range(2):
                                nc.tensor.matmul(
                                    ov[half][:, qlo:TQ],
                                    lhsT=va[:, 2 * p + half, :],
                                    rhs=es_pair[half][:, qlo:TQ],
                                    start=(kt == 0),
                                    stop=(kt == kmax - 1),
                                )

                        prev = None
                        for kt in range(kmax):
                            # diagonal tiles only contribute to q >= k: narrow
                            # the S-matmul/exp/mask/V to the valid q-range
                            d = kt - 4 * jq
                            qlo = 128 * d if d >= 0 else 0
                            es_pair = []
                            for half in range(2):
                                r = 64 * half
                                sps = spool[half].tile(
                                    [128, TQ], F32, tag="s", name="sps"
                                )
                                nc.tensor.matmul(
                                    sps[:, qlo:TQ],
                                    lhsT=kT[p][
                                        r : r + 64, kt * 128 : (kt + 1) * 128
                                    ],
                                    rhs=qT[p][
                                        r : r + 64,
                                        jq * TQ + qlo : (jq + 1) * TQ,
                                    ],
                                    start=True,
                                    stop=True,
                                )
                                es = esp.tile([128, TQ], ATT_DT, tag="es", name="es")
                                nc.scalar.activation(
                                    out=es[:, qlo:TQ],
                                    in_=sps[:, qlo:TQ],
                                    func=mybir.ActivationFunctionType.Exp,
                                    scale=SCALE,
                                )
                                if d >= 0:
                                    nc.vector.tensor_mul(
                                        out=es[:, qlo:TQ],
                                        in0=es[:, qlo:TQ],
                                        in1=mask[:, 384 : 384 + TQ - qlo],
                                    )
                                es_pair.append(es)
                            if prev is not None:
                                emit_v(*prev)
                            prev = (kt, qlo, es_pair)
                        emit_v(*prev)
                        # epilogue: move unnormalized O and denominators out
                        for half in range(2):
                            nc.vector.tensor_copy(
                                out=oT[p][
                                    64 * half : 64 * half + 64,
                                    jq * TQ : (jq + 1) * TQ,
                                ],
                                in_=ov[half][0:64, :],
                            )
                            dt_t = dtp.tile([1, TQ], F32, tag="dt", name="dt")
                            nc.vector.tensor_copy(
                                out=dt_t[:], in_=ov[half][64:65, :]
                            )
                            nc.sync.dma_start(
                                out=den4[2 * p + half : 2 * p + half + 1, :],
                                in_=dt_t[:],
                            )
                    rec4 = dtp.tile([4, TQ], F32R, tag="rec4", name="rec4")
                    nc.vector.reciprocal(out=rec4[:], in_=den4[:])
                    rp_ts = []
                    for p in range(2):
                        rp_t = rpp.tile([2, TQ], F32R, tag="rp", name="rp")
                        nc.sync.dma_start(
                            out=rp_t[:], in_=rec4[2 * p : 2 * p + 2, :]
                        )
                        rp_ts.append(rp_t)
                    rp_map[jq] = rp_ts

                def emit_tail(jq):
                    # normalize this column block (both pairs)
                    for p in range(2):
                        rp_t = rp_map[jq][p]
                        recb = pxp.tile([128, TQ], F32, tag="x", name="recb")
                        nc.tensor.matmul(
                            recb[:],
                            lhsT=bc2_sb[:],
                            rhs=rp_t[:],
                            start=True,
                            stop=True,
                        )
                        dst = oT[p][:, jq * TQ : (jq + 1) * TQ]
                        nc.vector.tensor_mul(out=dst, in0=dst, in1=recb[:])

                    # projection for this column block
                    for et in range(C // 128):
                        ps = pxp.tile([128, TQ], F32, tag="x", name="pmm")
                        for p in range(2):
                            nc.tensor.matmul(
                                ps[:],
                                lhsT=wp_sb[p][:, et * 128 : (et + 1) * 128],
                                rhs=oT[p][:, jq * TQ : (jq + 1) * TQ],
                                start=(p == 0),
                                stop=(p == 1),
                            )
                        o = ocp.tile([128, TQ], F32, tag="oc", name="oc")
                        nc.vector.tensor_copy(out=o[:], in_=ps[:])
                        nc.sync.dma_start(
                            out=rs_in[jq][et * 128 : (et + 1) * 128, :], in_=o[:]
                        )
                    # reduce-scatter this column block across the batch group
                    nc.gpsimd.collective_compute(
                        "ReduceScatter",
                        mybir.AluOpType.add,
                        ins=[rs_in[jq][:]],
                        outs=[rs_out[jq][:]],
                        replica_groups=GROUPS,
                    )

                for jq in range(NJQ):
                    emit_attention(jq)
                    if jq >= 1:
                        emit_tail(jq - 1)
                emit_tail(NJQ - 1)

                # ---- final: bias + output -----------------------------
                for i in range(2):
                    t = rsyp.tile([128, T], F32, tag="rs", name="rst")
                    for j in range(NJQ):
                        nc.sync.dma_start(
                            out=t[:, j * TQ : (j + 1) * TQ],
                            in_=rs_out[j][i * 128 : (i + 1) * 128, :],
                        )
                    nc.vector.tensor_scalar_add(
                        out=t[:], in0=t[:], scalar1=bp_sb[i][:]
                    )
                    nc.sync.dma_start(
                        out=y_d[i * 128 : (i + 1) * 128, :], in_=t[:]
                    )


    nc.compile()
    return nc


def _get_program():
    global _PROG
    if _PROG is None:
        _PROG = _build_program()
    return _PROG


def kernel(x, W_qkv, b_qkv, W_proj, b_proj):
    x = np.asarray(x, dtype=np.float32)
    W_qkv = np.asarray(W_qkv, dtype=np.float32)
    b_qkv = np.asarray(b_qkv, dtype=np.float32)
    W_proj = np.asarray(W_proj, dtype=np.float32)
    b_proj = np.asarray(b_proj, dtype=np.float32)

    nc = _get_program()

    mm_np = np.float16
    att_np = np.float16
    u = np.arange(896)[None, :]
    kl = np.arange(128)[:, None]
    mask_host = (u >= kl + 384).astype(att_np)
    ones_host = np.ones((128, 64), dtype=np.float32)
    onesb_host = np.ones((128, 64), dtype=att_np)
    bc2_host = np.zeros((2, 128), dtype=np.float32)
    bc2_host[0, 0:64] = 1.0
    bc2_host[1, 64:128] = 1.0

    xts = [np.ascontiguousarray(x[b].T).astype(mm_np) for b in range(B)]
    in_maps = []
    for c in range(N_CORES):
        b, g = divmod(c, 4)
        cs = slice(g * G * DH, (g + 1) * G * DH)
        r = c % 4
        in_maps.append(
            {
                "xt": xts[b],
                "wq": np.ascontiguousarray(W_qkv[:, cs]).astype(mm_np),
                "wk": np.ascontiguousarray(W_qkv[:, C:][:, cs]).astype(mm_np),
                "wv": np.ascontiguousarray(W_qkv[:, 2 * C :][:, cs]).astype(mm_np),
                "wp": np.ascontiguousarray(W_proj[cs, :]).astype(mm_np),
                "bq": np.ascontiguousarray(b_qkv[cs]).reshape(-1, 1),
                "bk": np.ascontiguousarray(b_qkv[C:][cs]).reshape(-1, 1),
                "bv": np.ascontiguousarray(b_qkv[2 * C :][cs]).reshape(1, -1),
                "bp": np.ascontiguousarray(
                    b_proj[r * 256 : (r + 1) * 256]
                ).reshape(-1, 1),
                "mask": mask_host,
                "ones": ones_host,
                "onesb": onesb_host,
                "bc2": bc2_host,
            }
        )

    global _last_in_maps
    _last_in_maps = in_maps
    res = run_bass_kernel_spmd(nc, in_maps, list(range(N_CORES)))

    y = np.empty((B, T, C), dtype=np.float32)
    for b in range(B):
        yT = np.concatenate(
            [res.results[4 * b + r]["y"] for r in range(4)], axis=0
        )
        y[b] = yT.T
    return y



# revision 6
# speedup vs baseline: 1.1850x; 1.1850x over previous
"""Causal self-attention on 8 Trainium2 NeuronCores.

Reference (fp32):
    qkv = x @ W_qkv + b_qkv ; split q,k,v ; heads H=16, Dh=64
    scores = q @ k^T / sqrt(Dh), causal mask, softmax
    out = (attn @ v) re-merged ; y = out @ W_proj + b_proj

Sharding: tensor-parallel over heads x data-parallel over batch.
Core c (0..7) owns batch b = c//4 and head group g = c%4 (heads 4g..4g+3).
Each core computes q^T,k^T,v for its 4 heads from x[b]^T, runs causal
attention (scores transposed layout, exp without max-subtraction -- scores
are O(5) so fp32 exp is safe, denominator via an appended ones-column in
the V matmul). The normalized per-head outputs O^T are exchanged with an
AllToAll across the 4 cores of the batch (each core keeps a 128-column
slice of the full O^T per 512-column block), after which every core
computes its own disjoint y^T columns with the full W_proj + b_proj --
no reduction collective needed. Host reassembles the column slices.

Emission interleaves the qkv projection (phase A), attention, and the
output projection per 512-column block so the scalar-engine exp stream
overlaps the PE-heavy projections and the PE never idles long enough to
drop out of the warm clock state. Matmuls run fp16 (full PE speed, 8x
finer mantissa than bf16); end-to-end error vs the fp32 reference is
~5e-4 of max|y|.
"""

import numpy as np

import concourse.bacc as bacc
import concourse.mybir as mybir
import concourse.tile as tile
from concourse.bass_utils import run_bass_kernel_spmd

B = 2
T = 2048
C = 1024
H = 16
DH = 64
G = 4  # heads per core
N_CORES = 8
TQ = 512  # q-chunk width
NKT = T // 128  # k tiles per head
NJQ = T // TQ  # q chunks
NCK = C // 128  # contraction tiles over model dim
SCALE = 1.0 / np.sqrt(DH)
GROUPS = [[0, 1, 2, 3], [4, 5, 6, 7]]

F32 = mybir.dt.float32
FP16 = mybir.dt.float16
MM_DT = FP16
ATT_DT = FP16

_PROG = None


def _build_program():
    nc = bacc.Bacc(
        "TRN2", target_bir_lowering=False, debug=False, num_devices=N_CORES
    )
    xt_d = nc.dram_tensor("xt", [C, T], MM_DT, kind="ExternalInput").ap()
    wq_d = nc.dram_tensor("wq", [C, G * DH], MM_DT, kind="ExternalInput").ap()
    wk_d = nc.dram_tensor("wk", [C, G * DH], MM_DT, kind="ExternalInput").ap()
    wv_d = nc.dram_tensor("wv", [C, G * DH], MM_DT, kind="ExternalInput").ap()
    wp_d = nc.dram_tensor("wp", [C, 2 * 128], MM_DT, kind="ExternalInput").ap()
    bq_d = nc.dram_tensor("bq", [G * DH, 1], F32, kind="ExternalInput").ap()
    bk_d = nc.dram_tensor("bk", [G * DH, 1], F32, kind="ExternalInput").ap()
    bv_d = nc.dram_tensor("bv", [1, G * DH], F32, kind="ExternalInput").ap()
    bp_d = nc.dram_tensor("bp", [2 * 128, 1], F32, kind="ExternalInput").ap()
    mask2_d = nc.dram_tensor("mask2", [128, 256], ATT_DT, kind="ExternalInput").ap()
    bc2_d = nc.dram_tensor("bc2", [2, 128], MM_DT, kind="ExternalInput").ap()
    onesb_d = nc.dram_tensor("onesb", [128, G], ATT_DT, kind="ExternalInput").ap()
    onesr_d = nc.dram_tensor("onesr", [1, 128], MM_DT, kind="ExternalInput").ap()
    ag_in = [
        nc.dram_tensor(f"ag_in{j}", [G * DH, TQ], ATT_DT).ap() for j in range(NJQ)
    ]
    ag_out = [
        nc.dram_tensor(f"ag_out{j}", [C, TQ], ATT_DT).ap() for j in range(NJQ)
    ]
    y_d = nc.dram_tensor("y", [2 * 128, T], F32, kind="ExternalOutput").ap()

    with tile.TileContext(nc) as tc:
        with (
            nc.allow_low_precision(reason="fp16 matmul pipeline by design"),
            tc.tile_pool(name="ll", bufs=1) as ll,
            tc.tile_pool(name="mm1", bufs=2, space="PSUM") as mm1,
            tc.tile_pool(name="spp", bufs=2, space="PSUM") as spp,
            tc.tile_pool(name="ovp", bufs=2, space="PSUM") as ovp,
            tc.tile_pool(name="esp", bufs=4) as esp,
            tc.tile_pool(name="dtp", bufs=4) as dtp,
            tc.tile_pool(name="rpp", bufs=4) as rpp,
            tc.tile_pool(name="oip", bufs=2) as oip,
            tc.tile_pool(name="yop", bufs=2) as yop,
        ):
            # ---- long-lived tiles -------------------------------------
            qT = [ll.tile([128, T], ATT_DT, tag=f"qT{p}", name=f"qT{p}") for p in range(2)]
            kT = [ll.tile([128, T], ATT_DT, tag=f"kT{p}", name=f"kT{p}") for p in range(2)]
            oT = [ll.tile([128, T], ATT_DT, tag=f"oT{p}", name=f"oT{p}") for p in range(2)]
            vaug = [ll.tile([128, G * 65], ATT_DT, tag=f"va{t}", name=f"va{t}") for t in range(NKT)]

            # small constants on the gpsimd queue (it is empty early on)
            mask2 = ll.tile([128, 256], ATT_DT, tag="mask2")
            nc.gpsimd.dma_start(out=mask2[:], in_=mask2_d[:])
            bc2_sb = ll.tile([2, 128], MM_DT, tag="bc2")
            nc.gpsimd.dma_start(out=bc2_sb[:], in_=bc2_d[:])
            onesr_sb = ll.tile([1, 128], MM_DT, tag="onesr")
            nc.gpsimd.dma_start(out=onesr_sb[:], in_=onesr_d[:])
            bq_sb = [ll.tile([128, 1], F32, tag=f"bq{p}", name=f"bq{p}") for p in range(2)]
            bk_sb = [ll.tile([128, 1], F32, tag=f"bk{p}", name=f"bk{p}") for p in range(2)]
            for p in range(2):
                nc.gpsimd.dma_start(
                    out=bq_sb[p][:], in_=bq_d[p * 128 : (p + 1) * 128, :]
                )
                nc.gpsimd.dma_start(
                    out=bk_sb[p][:], in_=bk_d[p * 128 : (p + 1) * 128, :]
                )
            bv_sb = ll.tile([1, G * DH], F32, tag="bv")
            nc.gpsimd.dma_start(out=bv_sb[:], in_=bv_d[:])
            bp_sb = [ll.tile([128, 1], F32, tag=f"bp{i}", name=f"bp{i}") for i in range(2)]
            for i in range(2):
                nc.gpsimd.dma_start(
                    out=bp_sb[i][:], in_=bp_d[i * 128 : (i + 1) * 128, :]
                )

            # x^T column chunk j=0 + qkv weights, k-interleaved across the
            # sync and gpsimd queues so the first projection chains unblock
            # as early as possible
            wq_sb, wk_sb, wv_sb = [], [], []
            xt_sb = [
                ll.tile([128, T], MM_DT, tag=f"xt{k}", name=f"xt{k}")
                for k in range(NCK)
            ]
            for k in range(NCK):
                nc.sync.dma_start(
                    out=xt_sb[k][:, 0:TQ],
                    in_=xt_d[k * 128 : (k + 1) * 128, 0:TQ],
                )
                for name, dst, src in (
                    ("q", wq_sb, wq_d),
                    ("k", wk_sb, wk_d),
                    ("v", wv_sb, wv_d),
                ):
                    t = ll.tile([128, G * DH], MM_DT, tag=f"w{name}{k}", name=f"w{name}{k}")
                    nc.gpsimd.dma_start(
                        out=t[:], in_=src[k * 128 : (k + 1) * 128, :]
                    )
                    dst.append(t)
            for j in range(1, NJQ):
                for k in range(NCK):
                    eng = (nc.sync, nc.scalar)[k % 2]
                    eng.dma_start(
                        out=xt_sb[k][:, j * TQ : (j + 1) * TQ],
                        in_=xt_d[k * 128 : (k + 1) * 128, j * TQ : (j + 1) * TQ],
                    )
            # this core's W_proj column slice (needed from the first
            # projection onwards)
            wp_sb = [
                ll.tile([128, 2 * 128], MM_DT, tag=f"wp{k}", name=f"wp{k}")
                for k in range(NCK)
            ]
            for k in range(NCK):
                nc.gpsimd.dma_start(
                    out=wp_sb[k][:], in_=wp_d[k * 128 : (k + 1) * 128, :]
                )

            # bv broadcast across partitions (via ones-row matmul)
            bv_r = ll.tile([1, G * DH], MM_DT, tag="bvr")
            nc.vector.tensor_copy(out=bv_r[:], in_=bv_sb[:])
            bvb_ps = mm1.tile([128, G * DH], F32, tag="mm1", name="bvbps")
            bvb_sb = ll.tile([128, G * DH], F32, tag="bvb")
            nc.tensor.matmul(
                bvb_ps[:], lhsT=onesr_sb[:], rhs=bv_r[:], start=True, stop=True
            )
            nc.vector.tensor_copy(out=bvb_sb[:], in_=bvb_ps[:])

            mask3 = mask2.rearrange("p (h c) -> p h c", c=128)

            # ---- phase A: qkv projections for one 512-column block ----
            def emit_phase_a(j):
                for wsb, bsb, dst, use_act in (
                    (wq_sb, bq_sb, qT, True),
                    (wk_sb, bk_sb, kT, False),
                ):
                    for p in range(2):
                        ps = mm1.tile([128, TQ], F32, tag="mm1", name="qkps")
                        for k in range(NCK):
                            nc.tensor.matmul(
                                ps[:],
                                lhsT=wsb[k][:, p * 128 : (p + 1) * 128],
                                rhs=xt_sb[k][:, j * TQ : (j + 1) * TQ],
                                start=(k == 0),
                                stop=(k == NCK - 1),
                            )
                        dst_ap = dst[p][:, j * TQ : (j + 1) * TQ]
                        if use_act:
                            nc.scalar.add(out=dst_ap, in_=ps[:], add=bsb[p][:])
                        else:
                            nc.vector.tensor_scalar_add(
                                out=dst_ap, in0=ps[:], scalar1=bsb[p][:]
                            )
                for t in range(4 * j, 4 * j + 4):
                    ps = mm1.tile([128, G * DH], F32, tag="mm1", name="vps")
                    for k in range(NCK):
                        nc.tensor.matmul(
                            ps[:],
                            lhsT=xt_sb[k][:, t * 128 : (t + 1) * 128],
                            rhs=wv_sb[k][:],
                            start=(k == 0),
                            stop=(k == NCK - 1),
                        )
                    va = vaug[t].rearrange("p (h x) -> p h x", x=65)
                    nc.vector.tensor_add(
                        out=va[:, :, 0:64],
                        in0=ps[:].rearrange("p (h x) -> p h x", x=64),
                        in1=bvb_sb[:].rearrange("p (h x) -> p h x", x=64),
                    )
                    nc.sync.dma_start(
                        out=va[:, :, 64:65],
                        in_=onesb_d[:, 0:G].rearrange("p (h x) -> p h x", x=1),
                    )

            # ---- attention for one pair of heads over one q block -----
            den_map = {}

            def emit_attention_pair(jq, p):
                kmax = 4 * jq + 4
                if p == 0:
                    den_map[jq] = dtp.tile([4, TQ], F32, tag="den4", name="den4")
                den4 = den_map[jq]
                ov = [
                    ovp.tile([65, TQ], F32, tag="ov", name="ovA"),
                    ovp.tile([65, TQ], F32, tag="ov", name="ovB"),
                ]

                def emit_v(kt, qlo, es2):
                    va = vaug[kt].rearrange("p (h x) -> p h x", x=65)
                    for half in range(2):
                        nc.tensor.matmul(
                            ov[half][:, qlo:TQ],
                            lhsT=va[:, 2 * p + half, :],
                            rhs=es2[:, half * TQ + qlo : (half + 1) * TQ],
                            start=(kt == 0),
                            stop=(kt == kmax - 1),
                        )

                prev = None
                for kt in range(kmax):
                    # diagonal tiles only contribute to q >= k: narrow the
                    # S-matmul/exp/mask/V to the valid q-range
                    d = kt - 4 * jq
                    qlo = 128 * d if d >= 0 else 0
                    sps2 = spp.tile([128, 2 * TQ], F32, tag="s", name="sps2")
                    for half in range(2):
                        r = 64 * half
                        nc.tensor.matmul(
                            sps2[:, half * TQ + qlo : (half + 1) * TQ],
                            lhsT=kT[p][r : r + 64, kt * 128 : (kt + 1) * 128],
                            rhs=qT[p][r : r + 64, jq * TQ + qlo : (jq + 1) * TQ],
                            start=True,
                            stop=True,
                        )
                    es2 = esp.tile([128, 2 * TQ], ATT_DT, tag="es", name="es2")
                    s3 = sps2.rearrange("p (h q) -> p h q", q=TQ)
                    e3 = es2.rearrange("p (h q) -> p h q", q=TQ)
                    nc.scalar.activation(
                        out=e3[:, :, qlo:TQ],
                        in_=s3[:, :, qlo:TQ],
                        func=mybir.ActivationFunctionType.Exp,
                        scale=SCALE,
                    )
                    if d >= 0:
                        # causal mask is only non-trivial on the 128-column
                        # band that straddles the diagonal
                        nc.vector.tensor_mul(
                            out=e3[:, :, qlo : qlo + 128],
                            in0=e3[:, :, qlo : qlo + 128],
                            in1=mask3[:],
                        )
                    if prev is not None:
                        emit_v(*prev)
                    prev = (kt, qlo, es2)
                emit_v(*prev)
                # epilogue: move unnormalized O and denominators out
                for half in range(2):
                    nc.vector.tensor_copy(
                        out=oT[p][
                            64 * half : 64 * half + 64,
                            jq * TQ : (jq + 1) * TQ,
                        ],
                        in_=ov[half][0:64, :],
                    )
                    dt_t = dtp.tile([1, TQ], F32, tag="dt", name="dt")
                    nc.vector.tensor_copy(out=dt_t[:], in_=ov[half][64:65, :])
                    nc.sync.dma_start(
                        out=den4[2 * p + half : 2 * p + half + 1, :],
                        in_=dt_t[:],
                    )

            def emit_attention_finish(jq):
                den4 = den_map[jq]
                rec4 = dtp.tile([4, TQ], ATT_DT, tag="rec4", name="rec4")
                nc.vector.reciprocal(out=rec4[:], in_=den4[:])
                for p in range(2):
                    rp_t = rpp.tile([2, TQ], MM_DT, tag="rp", name="rp")
                    nc.sync.dma_start(out=rp_t[:], in_=rec4[2 * p : 2 * p + 2, :])
                    recb = mm1.tile([128, TQ], F32, tag="mm1", name="recb")
                    nc.tensor.matmul(
                        recb[:], lhsT=bc2_sb[:], rhs=rp_t[:], start=True, stop=True
                    )
                    dst = oT[p][:, jq * TQ : (jq + 1) * TQ]
                    nc.vector.tensor_mul(out=dst, in0=dst, in1=recb[:])
                    nc.sync.dma_start(
                        out=ag_in[jq][p * 128 : (p + 1) * 128, :],
                        in_=oT[p][:, jq * TQ : (jq + 1) * TQ],
                    )
                nc.gpsimd.collective_compute(
                    "AllGather",
                    mybir.AluOpType.bypass,
                    ins=[ag_in[jq][:]],
                    outs=[ag_out[jq][:]],
                    replica_groups=GROUPS,
                )

            # ---- output projection for one 128-column slice -----------
            oin_map = {}

            def emit_proj(jq, et):
                if et == 0:
                    oin = oip.tile([128, NCK * TQ], MM_DT, tag="oin", name="oin")
                    for ko in range(NCK):
                        nc.sync.dma_start(
                            out=oin[:, ko * TQ : (ko + 1) * TQ],
                            in_=ag_out[jq][ko * 128 : (ko + 1) * 128, :],
                        )
                    oin_map[jq] = oin
                oin = oin_map[jq]
                ps = mm1.tile([128, TQ], F32, tag="mm1", name="pps")
                for ko in range(NCK):
                    nc.tensor.matmul(
                        ps[:],
                        lhsT=wp_sb[ko][:, et * 128 : (et + 1) * 128],
                        rhs=oin[:, ko * TQ : (ko + 1) * TQ],
                        start=(ko == 0),
                        stop=(ko == NCK - 1),
                    )
                yo = yop.tile([128, TQ], F32, tag="yo", name="yo")
                nc.vector.tensor_scalar_add(
                    out=yo[:], in0=ps[:], scalar1=bp_sb[et][:]
                )
                nc.sync.dma_start(
                    out=y_d[et * 128 : (et + 1) * 128, jq * TQ : (jq + 1) * TQ],
                    in_=yo[:],
                )

            # ---- main emission: interleave per 512-column block -------
            for jq in range(NJQ):
                emit_phase_a(jq)
                emit_attention_pair(jq, 0)
                if jq >= 1:
                    emit_proj(jq - 1, 0)
                emit_attention_pair(jq, 1)
                emit_attention_finish(jq)
                if jq >= 1:
                    emit_proj(jq - 1, 1)
            emit_proj(NJQ - 1, 0)
            emit_proj(NJQ - 1, 1)

    nc.compile()
    return nc


def _get_program():
    global _PROG
    if _PROG is None:
        _PROG = _build_program()
    return _PROG


def kernel(x, W_qkv, b_qkv, W_proj, b_proj):
    x = np.asarray(x, dtype=np.float32)
    W_qkv = np.asarray(W_qkv, dtype=np.float32)
    b_qkv = np.asarray(b_qkv, dtype=np.float32)
    W_proj = np.asarray(W_proj, dtype=np.float32)
    b_proj = np.asarray(b_proj, dtype=np.float32)

    nc = _get_program()

    mm_np = np.float16
    att_np = np.float16
    kl = np.arange(128)[:, None]
    jj = np.arange(128)[None, :]
    band = (jj >= kl).astype(att_np)
    mask2_host = np.concatenate([band, band], axis=1)
    bc2_host = np.zeros((2, 128), dtype=mm_np)
    bc2_host[0, 0:64] = 1.0
    bc2_host[1, 64:128] = 1.0
    onesb_host = np.ones((128, G), dtype=att_np)
    onesr_host = np.ones((1, 128), dtype=mm_np)

    xts = [np.ascontiguousarray(x[b].T).astype(mm_np) for b in range(B)]
    in_maps = []
    for c in range(N_CORES):
        b, g = divmod(c, 4)
        cs = slice(g * G * DH, (g + 1) * G * DH)
        in_maps.append(
            {
                "xt": xts[b],
                "wq": np.ascontiguousarray(W_qkv[:, cs]).astype(mm_np),
                "wk": np.ascontiguousarray(W_qkv[:, C:][:, cs]).astype(mm_np),
                "wv": np.ascontiguousarray(W_qkv[:, 2 * C :][:, cs]).astype(mm_np),
                "wp": np.ascontiguousarray(W_proj[:, cs]).astype(mm_np),
                "bq": np.ascontiguousarray(b_qkv[cs]).reshape(-1, 1),
                "bk": np.ascontiguousarray(b_qkv[C:][cs]).reshape(-1, 1),
                "bv": np.ascontiguousarray(b_qkv[2 * C :][cs]).reshape(1, -1),
                "bp": np.ascontiguousarray(b_proj[cs]).reshape(-1, 1),
                "mask2": mask2_host,
                "bc2": bc2_host,
                "onesb": onesb_host,
                "onesr": onesr_host,
            }
        )

    global _last_in_maps
    _last_in_maps = in_maps
    res = run_bass_kernel_spmd(nc, in_maps, list(range(N_CORES)))

    y = np.empty((B, T, C), dtype=np.float32)
    for b in range(B):
        yT = np.concatenate(
            [res.results[4 * b + r]["y"] for r in range(4)], axis=0
        )
        y[b] = yT.T
    return y


# revision 8
# speedup vs baseline: 1.3750x; 1.1604x over previous
"""Causal self-attention on 8 Trainium2 NeuronCores.

Reference (fp32):
    qkv = x @ W_qkv + b_qkv ; split q,k,v ; heads H=16, Dh=64
    scores = q @ k^T / sqrt(Dh), causal mask, softmax
    out = (attn @ v) re-merged ; y = out @ W_proj + b_proj

Sharding: tensor-parallel over heads x data-parallel over batch.
Core c (0..7) owns batch b = c//4 and head group g = c%4 (heads 4g..4g+3,
organized as pairs p=0,1 of two heads each). Each core computes
q^T,k^T,v for its 4 heads from x[b]^T, runs causal attention (scores in
transposed layout, exp without max-subtraction -- scores are O(5) so
fp32 exp is safe, denominator via an appended ones-column in the V
matmul). As soon as a pair's output O^T block is normalized it is
AllGathered (fp16, 128KB) across the 4 cores of the batch; each core
then computes its own 256-row slice of y^T with a per-core
(row-permuted) W_proj column slice + bias. No reduction collective.

The emission interleaves everything at instruction granularity: the
qkv-projection chains for the next column block and the output
projections for the previous one are spread between the attention
steps of the current block, so the tensor engine never sits behind an
exp-gated matmul with an empty pipe (which would drop it out of the
warm HAM clock state), and the scalar engine's exp stream starts
within the first microseconds and never drains.

Matmuls run fp16 (full PE speed, 8x finer mantissa than bf16);
end-to-end error vs the fp32 reference is ~5e-4 of max|y|.
"""

import numpy as np

import concourse.bacc as bacc
import concourse.mybir as mybir
import concourse.tile as tile
from concourse.bass_utils import run_bass_kernel_spmd

B = 2
T = 2048
C = 1024
H = 16
DH = 64
G = 4  # heads per core
N_CORES = 8
TQ = 512  # q-chunk width
NKT = T // 128  # k tiles per head
NJQ = T // TQ  # q chunks
NCK = C // 128  # contraction tiles over model dim
SCALE = 1.0 / np.sqrt(DH)
GROUPS = [[0, 1, 2, 3], [4, 5, 6, 7]]

F32 = mybir.dt.float32
FP16 = mybir.dt.float16
MM_DT = FP16
ATT_DT = FP16

_PROG = None


def _build_program():
    nc = bacc.Bacc(
        "TRN2", target_bir_lowering=False, debug=False, num_devices=N_CORES
    )
    xt_d = nc.dram_tensor("xt", [C, T], MM_DT, kind="ExternalInput").ap()
    wq_d = nc.dram_tensor("wq", [C, G * DH], MM_DT, kind="ExternalInput").ap()
    wk_d = nc.dram_tensor("wk", [C, G * DH], MM_DT, kind="ExternalInput").ap()
    wv_d = nc.dram_tensor("wv", [C, G * DH], MM_DT, kind="ExternalInput").ap()
    # wp rows are pair-permuted on the host to match the AllGather's
    # rank-stacked row order: rows 0:512 belong to pair 0, 512:1024 pair 1
    wp_d = nc.dram_tensor("wp", [C, 2 * 128], MM_DT, kind="ExternalInput").ap()
    bq_d = nc.dram_tensor("bq", [G * DH, 1], F32, kind="ExternalInput").ap()
    bk_d = nc.dram_tensor("bk", [G * DH, 1], F32, kind="ExternalInput").ap()
    bv_d = nc.dram_tensor("bv", [1, G * DH], F32, kind="ExternalInput").ap()
    bp_d = nc.dram_tensor("bp", [2 * 128, 1], F32, kind="ExternalInput").ap()
    mask2_d = nc.dram_tensor("mask2", [128, 256], ATT_DT, kind="ExternalInput").ap()
    bc2_d = nc.dram_tensor("bc2", [2, 128], MM_DT, kind="ExternalInput").ap()
    onesb_d = nc.dram_tensor("onesb", [128, G], ATT_DT, kind="ExternalInput").ap()
    onesr_d = nc.dram_tensor("onesr", [1, 128], MM_DT, kind="ExternalInput").ap()
    ag_in = [
        [
            nc.dram_tensor(f"ag_in{j}_{p}", [128, TQ], ATT_DT).ap()
            for p in range(2)
        ]
        for j in range(NJQ)
    ]
    ag_out = [
        [
            nc.dram_tensor(f"ag_out{j}_{p}", [512, TQ], ATT_DT).ap()
            for p in range(2)
        ]
        for j in range(NJQ)
    ]
    y_d = nc.dram_tensor("y", [2 * 128, T], F32, kind="ExternalOutput").ap()

    with tile.TileContext(nc) as tc:
        with (
            nc.allow_low_precision(reason="fp16 matmul pipeline by design"),
            tc.tile_pool(name="ll", bufs=1) as ll,
            tc.tile_pool(name="mm1", bufs=2, space="PSUM") as mm1,
            tc.tile_pool(name="spp", bufs=2, space="PSUM") as spp,
            tc.tile_pool(name="ovp", bufs=2, space="PSUM") as ovp,
            tc.tile_pool(name="esp", bufs=4) as esp,
            tc.tile_pool(name="dtp", bufs=6) as dtp,
            tc.tile_pool(name="rpp", bufs=4) as rpp,
            tc.tile_pool(name="oip", bufs=4) as oip,
            tc.tile_pool(name="yop", bufs=2) as yop,
        ):
            # ---- long-lived tiles -------------------------------------
            qT = [ll.tile([128, T], ATT_DT, tag=f"qT{p}", name=f"qT{p}") for p in range(2)]
            kT = [ll.tile([128, T], ATT_DT, tag=f"kT{p}", name=f"kT{p}") for p in range(2)]
            oT = [ll.tile([128, T], ATT_DT, tag=f"oT{p}", name=f"oT{p}") for p in range(2)]
            vaug = [ll.tile([128, G * 65], ATT_DT, tag=f"va{t}", name=f"va{t}") for t in range(NKT)]

            # small constants first on the gpsimd queue (tiny, out of the way)
            mask2 = ll.tile([128, 256], ATT_DT, tag="mask2")
            nc.gpsimd.dma_start(out=mask2[:], in_=mask2_d[:])
            bc2_sb = ll.tile([2, 128], MM_DT, tag="bc2")
            nc.gpsimd.dma_start(out=bc2_sb[:], in_=bc2_d[:])
            onesr_sb = ll.tile([1, 128], MM_DT, tag="onesr")
            nc.gpsimd.dma_start(out=onesr_sb[:], in_=onesr_d[:])
            bq_sb = [ll.tile([128, 1], F32, tag=f"bq{p}", name=f"bq{p}") for p in range(2)]
            bk_sb = [ll.tile([128, 1], F32, tag=f"bk{p}", name=f"bk{p}") for p in range(2)]
            for p in range(2):
                nc.gpsimd.dma_start(
                    out=bq_sb[p][:], in_=bq_d[p * 128 : (p + 1) * 128, :]
                )
                nc.gpsimd.dma_start(
                    out=bk_sb[p][:], in_=bk_d[p * 128 : (p + 1) * 128, :]
                )
            bv_sb = ll.tile([1, G * DH], F32, tag="bv")
            nc.gpsimd.dma_start(out=bv_sb[:], in_=bv_d[:])
            bp_sb = [ll.tile([128, 1], F32, tag=f"bp{i}", name=f"bp{i}") for i in range(2)]
            for i in range(2):
                nc.gpsimd.dma_start(
                    out=bp_sb[i][:], in_=bp_d[i * 128 : (i + 1) * 128, :]
                )

            # x^T column chunk j=0 (sync) k-interleaved with the q weights
            # (gpsimd) so the first projection chain unblocks earliest
            wq_sb, wk_sb, wv_sb = [], [], []
            xt_sb = [
                ll.tile([128, T], MM_DT, tag=f"xt{k}", name=f"xt{k}")
                for k in range(NCK)
            ]
            for k in range(NCK):
                nc.sync.dma_start(
                    out=xt_sb[k][:, 0:TQ],
                    in_=xt_d[k * 128 : (k + 1) * 128, 0:TQ],
                )
                t = ll.tile([128, G * DH], MM_DT, tag=f"wq{k}", name=f"wq{k}")
                nc.gpsimd.dma_start(out=t[:], in_=wq_d[k * 128 : (k + 1) * 128, :])
                wq_sb.append(t)
            for k in range(NCK):
                for name, dst, src in (("k", wk_sb, wk_d), ("v", wv_sb, wv_d)):
                    t = ll.tile([128, G * DH], MM_DT, tag=f"w{name}{k}", name=f"w{name}{k}")
                    nc.gpsimd.dma_start(
                        out=t[:], in_=src[k * 128 : (k + 1) * 128, :]
                    )
                    dst.append(t)
            for j in range(1, NJQ):
                for k in range(NCK):
                    eng = (nc.sync, nc.scalar)[k % 2]
                    eng.dma_start(
                        out=xt_sb[k][:, j * TQ : (j + 1) * TQ],
                        in_=xt_d[k * 128 : (k + 1) * 128, j * TQ : (j + 1) * TQ],
                    )
            # pair-permuted W_proj column slice (first needed by proj(0))
            wp_sb = [
                ll.tile([128, 2 * 128], MM_DT, tag=f"wp{k}", name=f"wp{k}")
                for k in range(NCK)
            ]
            for k in range(NCK):
                nc.gpsimd.dma_start(
                    out=wp_sb[k][:], in_=wp_d[k * 128 : (k + 1) * 128, :]
                )

            # bv broadcast across partitions (via ones-row matmul)
            bv_r = ll.tile([1, G * DH], MM_DT, tag="bvr")
            nc.vector.tensor_copy(out=bv_r[:], in_=bv_sb[:])
            bvb_ps = mm1.tile([128, G * DH], F32, tag="mm1", name="bvbps")
            bvb_sb = ll.tile([128, G * DH], F32, tag="bvb")
            nc.tensor.matmul(
                bvb_ps[:], lhsT=onesr_sb[:], rhs=bv_r[:], start=True, stop=True
            )
            nc.vector.tensor_copy(out=bvb_sb[:], in_=bvb_ps[:])

            mask3 = mask2.rearrange("p (h c) -> p h c", c=128)

            # ---- phase A building blocks ------------------------------
            def emit_qk_chain(j, which, p):
                wsb, bsb, dst = (
                    (wq_sb, bq_sb, qT) if which == "q" else (wk_sb, bk_sb, kT)
                )
                ps = mm1.tile([128, TQ], F32, tag="mm1", name="qkps")
                for k in range(NCK):
                    nc.tensor.matmul(
                        ps[:],
                        lhsT=wsb[k][:, p * 128 : (p + 1) * 128],
                        rhs=xt_sb[k][:, j * TQ : (j + 1) * TQ],
                        start=(k == 0),
                        stop=(k == NCK - 1),
                    )
                nc.vector.tensor_scalar_add(
                    out=dst[p][:, j * TQ : (j + 1) * TQ],
                    in0=ps[:],
                    scalar1=bsb[p][:],
                )

            def emit_v_tile(t):
                ps = mm1.tile([128, G * DH], F32, tag="mm1", name="vps")
                for k in range(NCK):
                    nc.tensor.matmul(
                        ps[:],
                        lhsT=xt_sb[k][:, t * 128 : (t + 1) * 128],
                        rhs=wv_sb[k][:],
                        start=(k == 0),
                        stop=(k == NCK - 1),
                    )
                va = vaug[t].rearrange("p (h x) -> p h x", x=65)
                nc.vector.tensor_add(
                    out=va[:, :, 0:64],
                    in0=ps[:].rearrange("p (h x) -> p h x", x=64),
                    in1=bvb_sb[:].rearrange("p (h x) -> p h x", x=64),
                )
                nc.sync.dma_start(
                    out=va[:, :, 64:65],
                    in_=onesb_d[:, 0:G].rearrange("p (h x) -> p h x", x=1),
                )

            def phase_a_units(j, skip_v=()):
                units = []
                for which in ("q", "k"):
                    for p in range(2):
                        units.append(
                            lambda j=j, w=which, p=p: emit_qk_chain(j, w, p)
                        )
                for t in range(4 * j, 4 * j + 4):
                    if t not in skip_v:
                        units.append(lambda t=t: emit_v_tile(t))
                return units

            # ---- attention --------------------------------------------
            den_map = {}
            oin_map = {}

            def emit_pair_finish(jq, p):
                # normalize pair p of block jq and launch its AllGather
                den2 = den_map[(jq, p)]
                rec2 = dtp.tile([128, 8], ATT_DT, tag="rec2", name="rec2")
                nc.vector.reciprocal(out=rec2[:], in_=den2[:])
                rp_t = rpp.tile([2, TQ], MM_DT, tag="rp", name="rp")
                for half in range(2):
                    nc.sync.dma_start(
                        out=rp_t[half : half + 1, :],
                        in_=rec2[:, 4 * half : 4 * half + 4],
                    )
                recb = mm1.tile([128, TQ], F32, tag="mm1", name="recb")
                nc.tensor.matmul(
                    recb[:], lhsT=bc2_sb[:], rhs=rp_t[:], start=True, stop=True
                )
                dst = oT[p][:, jq * TQ : (jq + 1) * TQ]
                nc.vector.tensor_mul(out=dst, in0=dst, in1=recb[:])
                nc.sync.dma_start(out=ag_in[jq][p][:], in_=dst)
                nc.gpsimd.collective_compute(
                    "AllGather",
                    mybir.AluOpType.bypass,
                    ins=[ag_in[jq][p][:]],
                    outs=[ag_out[jq][p][:]],
                    replica_groups=GROUPS,
                )
                # stage the gathered O^T rows for the projection; these sit
                # right behind the AllGather's wait on the gpsimd queue
                oin = oip.tile([128, 4 * TQ], MM_DT, tag="oin", name="oin")
                for ko in range(4):
                    nc.gpsimd.dma_start(
                        out=oin[:, ko * TQ : (ko + 1) * TQ],
                        in_=ag_out[jq][p][ko * 128 : (ko + 1) * 128, :],
                    )
                oin_map[(jq, p)] = oin

            def emit_attention_pair(jq, p, fill):
                # S/exp/mask/V pipeline for pair p over q block jq. `fill`
                # is a list of closures (independent PE work) spread between
                # the attention steps so the PE pipe never drains while the
                # scalar engine works through the exp stream.
                kmax = 4 * jq + 4
                nf = len(fill)
                stride = max(1, (kmax - 3) // nf) if nf else kmax + 1
                fi = 0
                den2 = dtp.tile([128, 8], F32, tag="den2", name="den2")
                den_map[(jq, p)] = den2
                ov = [
                    ovp.tile([65, TQ], F32, tag="ov", name="ovA"),
                    ovp.tile([65, TQ], F32, tag="ov", name="ovB"),
                ]

                def emit_v(kt, qlo, es2):
                    va = vaug[kt].rearrange("p (h x) -> p h x", x=65)
                    for half in range(2):
                        nc.tensor.matmul(
                            ov[half][:, qlo:TQ],
                            lhsT=va[:, 2 * p + half, :],
                            rhs=es2[:, half * TQ + qlo : (half + 1) * TQ],
                            start=(kt == 0),
                            stop=(kt == kmax - 1),
                        )

                prev = None
                for kt in range(kmax):
                    # diagonal tiles only contribute to q >= k: narrow the
                    # S-matmul/exp/mask/V to the valid q-range
                    d = kt - 4 * jq
                    qlo = 128 * d if d >= 0 else 0
                    sps2 = spp.tile([128, 2 * TQ], F32, tag="s", name="sps2")
                    for half in range(2):
                        r = 64 * half
                        nc.tensor.matmul(
                            sps2[:, half * TQ + qlo : (half + 1) * TQ],
                            lhsT=kT[p][r : r + 64, kt * 128 : (kt + 1) * 128],
                            rhs=qT[p][r : r + 64, jq * TQ + qlo : (jq + 1) * TQ],
                            start=True,
                            stop=True,
                        )
                    es2 = esp.tile([128, 2 * TQ], ATT_DT, tag="es", name="es2")
                    s3 = sps2.rearrange("p (h q) -> p h q", q=TQ)
                    e3 = es2.rearrange("p (h q) -> p h q", q=TQ)
                    nc.scalar.activation(
                        out=e3[:, :, qlo:TQ],
                        in_=s3[:, :, qlo:TQ],
                        func=mybir.ActivationFunctionType.Exp,
                        scale=SCALE,
                    )
                    if d >= 0:
                        # causal mask is only non-trivial on the 128-column
                        # band that straddles the diagonal
                        nc.vector.tensor_mul(
                            out=e3[:, :, qlo : qlo + 128],
                            in0=e3[:, :, qlo : qlo + 128],
                            in1=mask3[:],
                        )
                    if prev is not None:
                        emit_v(*prev)
                    prev = (kt, qlo, es2)
                    if fi < nf and kt % stride == stride - 1:
                        fill[fi]()
                        fi += 1
                emit_v(*prev)
                while fi < nf:
                    fill[fi]()
                    fi += 1
                # epilogue: move unnormalized O out, pack denominators into
                # a lane-parallel [128, 8] layout for the reciprocal
                for half in range(2):
                    nc.vector.tensor_copy(
                        out=oT[p][
                            64 * half : 64 * half + 64,
                            jq * TQ : (jq + 1) * TQ,
                        ],
                        in_=ov[half][0:64, :],
                    )
                    dt_t = dtp.tile([1, TQ], F32, tag="dt", name="dt")
                    nc.vector.tensor_copy(out=dt_t[:], in_=ov[half][64:65, :])
                    nc.sync.dma_start(
                        out=den2[:, 4 * half : 4 * half + 4],
                        in_=dt_t[:],
                    )

            # ---- output projection (one 128-row slice of y^T) ---------
            def emit_proj(jq, et):
                ps = mm1.tile([128, TQ], F32, tag="mm1", name="pps")
                first = True
                for p in range(2):
                    oin = oin_map[(jq, p)]
                    for ko in range(4):
                        nc.tensor.matmul(
                            ps[:],
                            lhsT=wp_sb[4 * p + ko][:, et * 128 : (et + 1) * 128],
                            rhs=oin[:, ko * TQ : (ko + 1) * TQ],
                            start=first,
                            stop=(p == 1 and ko == 3),
                        )
                        first = False
                yo = yop.tile([128, TQ], F32, tag="yo", name="yo")
                nc.vector.tensor_scalar_add(
                    out=yo[:], in0=ps[:], scalar1=bp_sb[et][:]
                )
                nc.sync.dma_start(
                    out=y_d[et * 128 : (et + 1) * 128, jq * TQ : (jq + 1) * TQ],
                    in_=yo[:],
                )

            # ---- main emission ----------------------------------------
            # filler units available during attention of block jq:
            #   - phase A chains of block jq+1
            #   - projections of block jq-1 (their AllGathers completed
            #     during attention of blocks jq-1 -> jq)
            #   - the deferred pair-1 finish of block jq-1
            for u in phase_a_units(0):
                u()
            deferred = None
            for jq in range(NJQ):
                half0 = []
                if deferred is not None:
                    half0.append(deferred)
                    deferred = None
                if jq < NJQ - 1:
                    skip = () if jq < NJQ - 2 else (13, 14, 15)
                    half0.extend(phase_a_units(jq + 1, skip_v=skip))
                else:
                    half0.extend(
                        lambda t=t: emit_v_tile(t) for t in (13, 14, 15)
                    )
                half1 = []
                if jq >= 1:
                    half1.append(lambda j=jq - 1: emit_proj(j, 0))
                    half1.append(lambda j=jq - 1: emit_proj(j, 1))
                emit_attention_pair(jq, 0, half0)
                # pair-0 finish lands a few steps into pair 1 (after the
                # reciprocal chain has had time to produce rp)
                half1.insert(
                    min(1, len(half1)), lambda j=jq: emit_pair_finish(j, 0)
                )
                emit_attention_pair(jq, 1, half1)
                if jq < NJQ - 1:
                    deferred = lambda j=jq: emit_pair_finish(j, 1)
                else:
                    emit_pair_finish(jq, 1)
            emit_proj(NJQ - 1, 0)
            emit_proj(NJQ - 1, 1)

    nc.compile()
    return nc


def _get_program():
    global _PROG
    if _PROG is None:
        _PROG = _build_program()
    return _PROG


def kernel(x, W_qkv, b_qkv, W_proj, b_proj):
    x = np.asarray(x, dtype=np.float32)
    W_qkv = np.asarray(W_qkv, dtype=np.float32)
    b_qkv = np.asarray(b_qkv, dtype=np.float32)
    W_proj = np.asarray(W_proj, dtype=np.float32)
    b_proj = np.asarray(b_proj, dtype=np.float32)

    nc = _get_program()

    mm_np = np.float16
    att_np = np.float16
    kl = np.arange(128)[:, None]
    jj = np.arange(128)[None, :]
    band = (jj >= kl).astype(att_np)
    mask2_host = np.concatenate([band, band], axis=1)
    bc2_host = np.zeros((2, 128), dtype=mm_np)
    bc2_host[0, 0:64] = 1.0
    bc2_host[1, 64:128] = 1.0
    onesb_host = np.ones((128, G), dtype=att_np)
    onesr_host = np.ones((1, 128), dtype=mm_np)

    # W_proj rows permuted to the AllGather's rank-stacked pair order:
    # for pair p the gathered rows are [g0:(h=2p, h=2p+1), g1:(...), ...]
    perm = np.concatenate(
        [
            np.arange(64 * (4 * g + 2 * p + e), 64 * (4 * g + 2 * p + e) + 64)
            for p in range(2)
            for g in range(4)
            for e in range(2)
        ]
    )
    wp_perm = W_proj[perm, :]

    xts = [np.ascontiguousarray(x[b].T).astype(mm_np) for b in range(B)]
    in_maps = []
    for c in range(N_CORES):
        b, g = divmod(c, 4)
        cs = slice(g * G * DH, (g + 1) * G * DH)
        in_maps.append(
            {
                "xt": xts[b],
                "wq": np.ascontiguousarray(W_qkv[:, cs]).astype(mm_np),
                "wk": np.ascontiguousarray(W_qkv[:, C:][:, cs]).astype(mm_np),
                "wv": np.ascontiguousarray(W_qkv[:, 2 * C :][:, cs]).astype(mm_np),
                "wp": np.ascontiguousarray(wp_perm[:, cs]).astype(mm_np),
                "bq": np.ascontiguousarray(b_qkv[cs]).reshape(-1, 1),
                "bk": np.ascontiguousarray(b_qkv[C:][cs]).reshape(-1, 1),
                "bv": np.ascontiguousarray(b_qkv[2 * C :][cs]).reshape(1, -1),
                "bp": np.ascontiguousarray(b_proj[cs]).reshape(-1, 1),
                "mask2": mask2_host,
                "bc2": bc2_host,
                "onesb": onesb_host,
                "onesr": onesr_host,
            }
        )

    global _last_in_maps
    _last_in_maps = in_maps
    res = run_bass_kernel_spmd(nc, in_maps, list(range(N_CORES)))

    y = np.empty((B, T, C), dtype=np.float32)
    for b in range(B):
        yT = np.concatenate(
            [res.results[4 * b + r]["y"] for r in range(4)], axis=0
        )
        y[b] = yT.T
    return y
